# revision 1
# baseline (speedup 1.0000x reference)
"""Trainium2 Bass kernel for nn_ContactMapDistError.

Computes, for each batch element b:
    mean over active contact pairs (r,s) of
      min_{v in region r, w in region s} || g1[b,r,v] - g2[b,s,w] ||

Strategy
--------
Host (cheap, O(B*R*VR)):
  - gather region vertex subsets g1, g2 via rid_to_vid
  - build feature matrices so that a single K=5 matmul produces the full
    pairwise squared-distance matrix:
        d2(v,w) = [-2x,-2y,-2z,sq1,1]_v . [x',y',z',1,sq2]_w
  - finish the v-axis min (segmented, tiny), sqrt, contact-mask mean

Device (8 cores, SPMD; core i -> batch i//2, r-half i%2), raw bass
(explicit semaphores; ISA allows only one sync-wait per instruction):
  - PE: float32r matmuls produce d2 in PSUM, [128 v-lanes x 1536 w]
    tiles, double-buffered
  - DVE: grouped min-reduce over each s-region's 96 w columns
  - output: per-core [128, 18*48] minima over w per (v-lane, chunk, s)
"""

import sys

sys.path.insert(0, "/opt/trn_rl_repo")

import numpy as np

import concourse.bass as bass
import concourse.mybir as mybir
from concourse.bass_utils import run_bass_kernel_spmd

F32 = mybir.dt.float32
F32R = mybir.dt.float32r

B, N, R, VR = 4, 10475, 48, 96
NCORES = 8
RH = R // 2            # r-regions handled per core
V = RH * VR            # packed v columns per core = 2304
T = V // 128           # v-chunks of 128 partitions = 18
W = R * VR             # full w width = 4608
WC = 1536              # psum w-chunk (3 banks, 16 s-regions)
NWC = W // WC          # = 3
K = 5                  # contraction dim
NK = T * NWC           # total chunk count = 54
NPT = 2                # psum double buffer

_cache = {}


def _build():
    if "nc" in _cache:
        return _cache["nc"]
    nc = bass.Bass()
    ab = nc.declare_dram_parameter("ab", [K, V + W], F32R, isOutput=False)
    s1out = nc.declare_dram_parameter("s1out", [128, T * R], F32, isOutput=True)

    abt = nc.alloc_sbuf_tensor("abt", [K, V + W], F32R).ap()
    s1buf = nc.alloc_sbuf_tensor("s1buf", [128, T * R], F32).ap()
    pts = [nc.alloc_psum_tensor(f"pt{i}", [128, WC], F32).ap() for i in range(NPT)]

    lt = abt[:, 0:V]
    rt = abt[:, V : V + W]

    with (
        nc.Block() as block,
        nc.semaphore("dma_sem") as dma_sem,
        nc.semaphore("pe_sem") as pe_sem,
        nc.semaphore("dve_sem") as dve_sem,
    ):

        @block.gpsimd
        def _(g):
            g.dma_start(abt, ab[:]).then_inc(dma_sem, 16)
            g.wait_ge(dve_sem, NK)
            g.dma_start(s1out[:], s1buf).then_inc(dma_sem, 16)
            g.wait_ge(dma_sem, 32)

        @block.tensor
        def _(pe):
            pe.wait_ge(dma_sem, 16)
            k = 0
            for t in range(T):
                for c in range(NWC):
                    if k >= NPT:
                        pe.wait_ge(dve_sem, k - NPT + 1)
                    pt = pts[k % NPT]
                    last = None
                    for m in range(WC // 512):
                        last = pe.matmul(
                            pt[:, m * 512 : (m + 1) * 512],
                            lt[:, t * 128 : (t + 1) * 128],
                            rt[:, c * WC + m * 512 : c * WC + (m + 1) * 512],
                            start=True,
                            stop=True,
                        )
                    last.then_inc(pe_sem)
                    k += 1

        @block.vector
        def _(v):
            k = 0
            for t in range(T):
                for c in range(NWC):
                    v.wait_ge(pe_sem, k + 1)
                    v.tensor_reduce(
                        s1buf[:, t * R + c * 16 : t * R + (c + 1) * 16],
                        pts[k % NPT].rearrange("p (g v) -> p g v", v=VR),
                        axis=mybir.AxisListType.X,
                        op=mybir.AluOpType.min,
                    ).then_inc(dve_sem)
                    k += 1

    _cache["nc"] = nc
    return nc


def _prep_inputs(v1s, v2s, rid_to_vid):
    """Build per-core fused lhsT|rhs feature matrices."""
    g1 = v1s[:, rid_to_vid, :]  # [B, R, VR, 3]
    g2 = v2s[:, rid_to_vid, :]
    g1_64 = g1.astype(np.float64)
    g2_64 = g2.astype(np.float64)
    sq1 = (g1_64 * g1_64).sum(-1)  # [B, R, VR]
    sq2 = (g2_64 * g2_64).sum(-1)

    in_maps = []
    for core in range(NCORES):
        b, h = divmod(core, 2)
        rs = slice(RH * h, RH * (h + 1))
        a = np.empty((K, V + W), np.float32)
        a[0:3, 0:V] = -2.0 * g1[b, rs].reshape(V, 3).T
        a[3, 0:V] = sq1[b, rs].reshape(V).astype(np.float32)
        a[4, 0:V] = 1.0
        a[0:3, V:] = g2[b].reshape(W, 3).T
        a[3, V:] = 1.0
        a[4, V:] = sq2[b].reshape(W).astype(np.float32)
        in_maps.append({"ab": a})
    return in_maps


def kernel(v1s, v2s, cmaps, rid_to_vid):
    v1s = np.asarray(v1s)
    v2s = np.asarray(v2s)
    cmaps = np.asarray(cmaps)
    rid_to_vid = np.asarray(rid_to_vid)

    nc = _build()
    in_maps = _prep_inputs(v1s, v2s, rid_to_vid)
    res = run_bass_kernel_spmd(nc, in_maps, core_ids=list(range(NCORES)))

    # assemble [B, R, R] min squared distances (r = person1 region rows)
    md2 = np.empty((B, R, R), np.float32)
    for core in range(NCORES):
        b, h = divmod(core, 2)
        out = res.results[core]["s1out"]  # [128, T*R]
        # [128, T, R] -> v = t*128 + p -> [V, R]
        per_v = out.reshape(128, T, R).transpose(1, 0, 2).reshape(V, R)
        # segmented min over each region's 96 rows
        md2[b, RH * h : RH * (h + 1), :] = per_v.reshape(RH, VR, R).min(axis=1)

    md = np.sqrt(np.maximum(md2, 0.0))
    m = cmaps.astype(np.float32)
    return ((md * m).sum(axis=(1, 2)) / m.sum(axis=(1, 2))).astype(np.float32)



# revision 2
# speedup vs baseline: 146.9352x; 146.9352x over previous
"""Masked per-core Bass kernel for nn_ContactMapDistError (v3).

Per core (b = core//2, 24-region half): for each region r, only the
s-regions with cmap[b,r,s]=1 (plus cheap bridging) are computed.
PE does [5,96]x[5,cols] fp32r matmuls per merged run of active s-groups;
the w-min reduce is split between DVE (grouped tensor_reduce, 8-group
psum chunks) and Pool (in-place tensor_tensor min tree, 16-group chunks).
Host does the final v-min, sqrt, and masked mean.

Because active patterns differ per core, each core gets its own compiled
program, dispatched concurrently on its own device.
"""

import sys

sys.path.insert(0, "/opt/trn_rl_repo")

import numpy as np

import concourse.bass as bass
import concourse.mybir as mybir

F32 = mybir.dt.float32
F32R = mybir.dt.float32r
F16 = mybir.dt.float16
MIN = mybir.AluOpType.min

B, N, R, VR = 4, 10475, 48, 96
NCORES = 8
RH = R // 2            # regions per core
V = RH * VR            # lhs columns per core = 2304
W = R * VR             # rhs columns = 4608
K = 7

POOL_G = 16            # groups per Pool chunk
DVE_G = 8              # groups per DVE chunk


def merge_runs(active):
    """Greedy run-merge over 48 slots: cover all active groups with runs,
    Returns list of (start, length) maximal runs (group indices).
    """
    idx = np.flatnonzero(active)
    if len(idx) == 0:
        return []
    # start with maximal runs of consecutive active groups
    runs = []
    s = p = idx[0]
    for i in idx[1:]:
        if i == p + 1:
            p = i
        else:
            runs.append([s, p - s + 1])
            s = p = i
    runs.append([s, p - s + 1])

    # fp16 matmuls have no small-piece penalty, so no bridging: cover
    # exactly the active groups with maximal runs.
    return [(s, L) for s, L in runs]


def make_plan(cmap_half):
    """cmap_half: [RH, R] bool for this core's regions.

    Returns dict with chunk list and output mapping.
    Each chunk: dict(engine='pool'|'dve', groups=[(region, s_group), ...],
                     segs=[(region, s_start, n_groups), ...])
    """
    # stream of (region, s_start, run_len) segments
    segs = []
    for r in range(RH):
        for s, L in merge_runs(cmap_half[r]):
            segs.append((r, s, L))

    # flatten to group stream, then cut into chunks with repeating pattern
    # [pool16, pool16, dve8, dve8] (2:1 group ratio matches engine rates)
    gstream = []
    for r, s, L in segs:
        for j in range(L):
            gstream.append((r, s + j))

    chunks = []
    pat = [("dve16", POOL_G), ("dve16", POOL_G), ("dve8", DVE_G)]
    pi = 0
    pos = 0
    while pos < len(gstream):
        eng, g = pat[pi % len(pat)]
        pi += 1
        take = gstream[pos : pos + g]
        pos += len(take)
        # consolidate contiguous groups back into segments
        csegs = []
        for r, s in take:
            if csegs and csegs[-1][0] == r and csegs[-1][1] + csegs[-1][2] == s:
                csegs[-1][2] += 1
            else:
                csegs.append([r, s, 1])
        chunks.append(
            dict(engine=eng, groups=take, segs=[tuple(x) for x in csegs])
        )
    if not gstream:
        # degenerate: no active pairs; compute one dummy group
        chunks.append(dict(engine="dve8", groups=[(0, 0)], segs=[(0, 0, 1)]))
        gstream = [(0, 0)]
    gtotal = len(gstream)
    return dict(chunks=chunks, gtotal=gtotal)


def build_core(plan, L=1):
    """Build the per-core Bass program for a chunk plan (DVE-only reduce).

    L > 1 repeats the full pipeline (including I/O DMAs) L times on-device;
    used for wall-clock L-loop timing of the steady-state iteration.
    """
    chunks = plan["chunks"]
    gtotal = plan["gtotal"]
    NC = len(chunks)

    nc = bass.Bass()
    ab = nc.declare_dram_parameter("ab", [K, V + W], F16, isOutput=False)
    s1out = nc.declare_dram_parameter("s1out", [96, gtotal], F32, isOutput=True)

    abt = nc.alloc_sbuf_tensor("abt", [K, V + W], F16).ap()
    s1buf = nc.alloc_sbuf_tensor("s1buf", [128, gtotal], F32).ap()
    slots = [
        nc.alloc_psum_tensor("pp0", [128, POOL_G * VR], F32).ap(),
        nc.alloc_psum_tensor("pp1", [128, POOL_G * VR], F32).ap(),
        nc.alloc_psum_tensor("pd", [128, DVE_G * VR], F32).ap(),
    ]

    lt = abt[:, 0:V]
    rt = abt[:, V : V + W]

    off = 0
    for i, c in enumerate(chunks):
        c["out_off"] = off
        off += len(c["groups"])
        c["idx"] = i
        c["slot"] = i % 3

    with (
        nc.Block() as block,
        nc.semaphore("dma_sem") as dma_sem,
        nc.semaphore("pe_dve") as pe_dve,
        nc.semaphore("dve_done") as dve_done,
    ):

        @block.sync
        def _(sp):
            for it in range(L):
                if it:
                    sp.wait_ge(dve_done, NC * it)
                sp.dma_start(abt[:, 0 : (V + W) // 2], ab[:, 0 : (V + W) // 2]) \
                    .then_inc(dma_sem, 16)
                sp.wait_ge(dve_done, NC * (it + 1))
                sp.dma_start(s1out[:], s1buf[0:96, :]).then_inc(dma_sem, 16)
            sp.wait_ge(dma_sem, 48 * L)

        @block.scalar
        def _(act):
            for it in range(L):
                if it:
                    act.wait_ge(dve_done, NC * it)
                act.dma_start(
                    abt[:, (V + W) // 2 : V + W], ab[:, (V + W) // 2 : V + W]
                ).then_inc(dma_sem, 16)

        @block.tensor
        def _(pe):
            for it in range(L):
                pe.wait_ge(dma_sem, 48 * it + 32)
                for c in chunks:
                    pt = slots[c["slot"]]
                    gi = NC * it + c["idx"]
                    if gi >= 3:
                        pe.wait_ge(dve_done, gi - 2)
                    last = None
                    ppos = 0
                    for r, s, Lr in c["segs"]:
                        cols = Lr * VR
                        done = 0
                        while done < cols:
                            take = min(512 - (ppos % 512), cols - done)
                            last = pe.matmul(
                                pt[0:96, ppos : ppos + take],
                                lt[:, r * VR : (r + 1) * VR],
                                rt[:, s * VR + done : s * VR + done + take],
                                start=True,
                                stop=True,
                            )
                            ppos += take
                            done += take
                    last.then_inc(pe_dve)

        @block.vector
        def _(v):
            for it in range(L):
                for c in chunks:
                    g = len(c["groups"])
                    v.wait_ge(pe_dve, NC * it + c["idx"] + 1)
                    v.tensor_reduce(
                        s1buf[0:96, c["out_off"] : c["out_off"] + g],
                        slots[c["slot"]][0:96, 0 : g * VR].rearrange(
                            "p (g v) -> p g v", v=VR
                        ),
                        axis=mybir.AxisListType.X,
                        op=MIN,
                    ).then_inc(dve_done)

    return nc


def prep_inputs(v1s, v2s, rid_to_vid):
    """Identical feature packing to the baseline kernel."""
    # round coordinates to fp16 first, then derive the squared norms from
    # the rounded values: the kernel then computes exact distances of the
    # perturbed points (products of fp16 are exact in f32 accumulation)
    g1 = v1s[:, rid_to_vid, :].astype(np.float16)
    g2 = v2s[:, rid_to_vid, :].astype(np.float16)
    g1_64 = g1.astype(np.float64)
    g2_64 = g2.astype(np.float64)
    sq1 = (g1_64 * g1_64).sum(-1)
    sq2 = (g2_64 * g2_64).sum(-1)

    in_maps = []
    for core in range(NCORES):
        b, h = divmod(core, 2)
        rs = slice(RH * h, RH * (h + 1))
        a = np.empty((K, V + W), np.float16)
        s1 = sq1[b, rs].reshape(V)
        s2 = sq2[b].reshape(W)
        s1_hi = s1.astype(np.float16).astype(np.float64)
        s2_hi = s2.astype(np.float16).astype(np.float64)
        a[0:3, 0:V] = -2.0 * g1[b, rs].reshape(V, 3).T
        a[3, 0:V] = s1_hi
        a[4, 0:V] = (s1 - s1_hi).astype(np.float16)
        a[5, 0:V] = 1.0
        a[6, 0:V] = 1.0
        a[0:3, V:] = g2[b].reshape(W, 3).T
        a[3, V:] = 1.0
        a[4, V:] = 1.0
        a[5, V:] = s2_hi
        a[6, V:] = (s2 - s2_hi).astype(np.float16)
        in_maps.append({"ab": a})
    return in_maps


def postprocess(results, plans, cmaps):
    """results[core]['s1out']: [96, gtotal]. Min over v, scatter, mean."""
    md2 = np.zeros((B, R, R), np.float32)
    for core in range(NCORES):
        b, h = divmod(core, 2)
        plan = plans[core]
        out = results[core]["s1out"]          # [96, gtotal]
        mins = out.min(axis=0)                # [gtotal]
        col = 0
        for c in plan["chunks"]:
            for r, s in c["groups"]:
                md2[b, RH * h + r, s] = mins[col]
                col += 1
    md = np.sqrt(np.maximum(md2, 0.0))
    m = cmaps.astype(np.float32)
    return ((md * m).sum(axis=(1, 2)) / m.sum(axis=(1, 2))).astype(np.float32)
